# revision 10
# baseline (speedup 1.0000x reference)
"""Trainium2 Bass kernel for nn_LocalAttention (T=4096, B=32, H=256, L=512, K=32).

Sharding: data-parallel over batch B across 8 cores (4 batch elements per core).

Per-core dataflow (h-on-partitions layout):
  1. coeff phase: kernel coeffs = lm @ Wk.T + bk computed on PE from a
     host-permuted Wk (bf16), landing as (4 batches, 8192) in PSUM chunks,
     copied to SBUF, bounced through DRAM to re-layout into (32 k, 128 h)
     conv weight tiles.
  2. global phase: global = lm @ Wg.T + bg on PE -> (128 h, 4 b) SBUF, used as
     the per-partition bias of the tanh activation.
  3. main stream per (b, hc, tt): conv = coeff.T @ toeplitz(prev) on PE into
     PSUM (128 h, 512 t); DVE adds encoded_contribution; ACT computes
     tanh(. + global_bias); PE reduces over h against Ws -> scores in
     (128 t-part, col) layout; softmax over T on-chip.
"""

import os
import sys

import numpy as np

if "/opt/trn_rl_repo" not in sys.path:
    sys.path.insert(0, "/opt/trn_rl_repo")

import ml_dtypes

T, B, H, L, K = 4096, 32, 256, 512, 32
NCORES = 8
BC = B // NCORES          # 4 batches per core
HCHUNKS = H // 128        # 2
TTILE = 512
NTT = T // TTILE          # 8 t-tiles per (b, hc)

_CACHE = {}


def _build_program(debug_dumps=False):
    import concourse.bacc as bacc
    import concourse.bass as bass
    import concourse.mybir as mybir
    import concourse.tile as tile
    from contextlib import ExitStack

    dt = mybir.dt
    fp32 = dt.float32
    bf16 = dt.bfloat16
    ts = bass.ts

    nc = bacc.Bacc(
        "TRN2",
        target_bir_lowering=False,
        debug=False,
        enable_asserts=False,
        num_devices=NCORES,
    )

    enc = nc.dram_tensor("enc", (BC, HCHUNKS, 128, T), fp32, kind="ExternalInput").ap()
    win = nc.dram_tensor("win", (BC, K, T), fp32, kind="ExternalInput").ap()
    wkp = nc.dram_tensor("wkp", (128, 4, 16, 512), bf16, kind="ExternalInput").ap()
    bkp = nc.dram_tensor("bkp", (1, K * H), bf16, kind="ExternalInput").ap()
    lmb = nc.dram_tensor("lmb", (128, 4, BC), bf16, kind="ExternalInput").ap()
    lmf = nc.dram_tensor("lmf", (128, 4, BC), fp32, kind="ExternalInput").ap()
    wgt = nc.dram_tensor("wgt", (128, 4, H), fp32, kind="ExternalInput").ap()
    bgp = nc.dram_tensor("bgp", (1, H), fp32, kind="ExternalInput").ap()
    wsv = nc.dram_tensor("wsv", (128, HCHUNKS), fp32, kind="ExternalInput").ap()
    msk = nc.dram_tensor("msk", (128, 128), fp32, kind="ExternalInput").ap()
    att = nc.dram_tensor("att", (128, 128), fp32, kind="ExternalOutput").ap()
    if debug_dumps:
        d_kernT = nc.dram_tensor("d_kernT", (BC, K * H), fp32, kind="ExternalOutput").ap()
        d_coef = nc.dram_tensor("d_coef", (K, BC * HCHUNKS, 128), fp32, kind="ExternalOutput").ap()
        d_glob = nc.dram_tensor("d_glob", (128, HCHUNKS, BC), fp32, kind="ExternalOutput").ap()
        d_hid = nc.dram_tensor("d_hid", (128, TTILE), fp32, kind="ExternalOutput").ap()
        d_tan = nc.dram_tensor("d_tan", (128, TTILE), fp32, kind="ExternalOutput").ap()
        d_sc = nc.dram_tensor("d_sc", (128, 128), fp32, kind="ExternalOutput").ap()
        d_exp = nc.dram_tensor("d_exp", (128, 128), fp32, kind="ExternalOutput").ap()

    TanhF = mybir.ActivationFunctionType.Tanh
    ExpF = mybir.ActivationFunctionType.Exp
    Add = mybir.AluOpType.add

    with tile.TileContext(nc) as tc, ExitStack() as ctx:
        # ---------- pools ----------
        coeff_pool = ctx.enter_context(tc.tile_pool(name="coeff", bufs=1))
        wk_pool = ctx.enter_context(tc.tile_pool(name="wkpool", bufs=6))
        small_pool = ctx.enter_context(tc.tile_pool(name="small", bufs=1))
        enc_pool = ctx.enter_context(tc.tile_pool(name="encp", bufs=6))
        win_pool = ctx.enter_context(tc.tile_pool(name="winp", bufs=2))
        hid_pool = ctx.enter_context(tc.tile_pool(name="hidp", bufs=4))
        tan_pool = ctx.enter_context(tc.tile_pool(name="tanp", bufs=4))
        psum_pool = ctx.enter_context(tc.tile_pool(name="psum", bufs=4, space="PSUM"))
        psum_cps = ctx.enter_context(tc.tile_pool(name="psumc", bufs=2, space="PSUM"))
        psum_misc = ctx.enter_context(tc.tile_pool(name="psumm", bufs=1, space="PSUM"))
        spsum_pool = ctx.enter_context(tc.tile_pool(name="spsum", bufs=1, space="PSUM"))
        dram_pool = ctx.enter_context(tc.tile_pool(name="dramp", bufs=1, space="DRAM"))

        # ---------- small input loads ----------
        lm_sb = small_pool.tile([128, 4, BC], bf16)
        nc.sync.dma_start(lm_sb[:], lmb)
        lmf_sb = small_pool.tile([128, 4, BC], fp32)
        nc.sync.dma_start(lmf_sb[:], lmf)
        wgt_sb = small_pool.tile([128, 4, H], fp32)
        nc.sync.dma_start(wgt_sb[:], wgt)
        bk_sb = small_pool.tile([1, K * H], bf16)
        nc.sync.dma_start(bk_sb[:], bkp)
        bg_sb = small_pool.tile([1, H], fp32)
        nc.sync.dma_start(bg_sb[:], bgp)
        ws_sb = small_pool.tile([128, HCHUNKS], fp32)
        nc.sync.dma_start(ws_sb[:], wsv)
        msk_sb = small_pool.tile([128, 128], fp32)
        nc.sync.dma_start(msk_sb[:], msk)

        one_b = small_pool.tile([1, BC], bf16)
        nc.vector.memset(one_b[:], 1.0)
        one_f = small_pool.tile([1, BC], fp32)
        nc.vector.memset(one_f[:], 1.0)
        ones128 = small_pool.tile([128, 1], fp32)
        nc.vector.memset(ones128[:], 1.0)
        ones1x128 = small_pool.tile([1, 128], fp32)
        nc.vector.memset(ones1x128[:], 1.0)

        # ---------- coeff phase: kernT[b, k*256+h] = (lm @ WkP + bk) ----------
        kernT_sb = coeff_pool.tile([BC, K * H], fp32)
        for j in range(16):
            cps = psum_cps.tile([BC, 512], fp32, tag="cps")
            for i in range(4):
                wk_sb = wk_pool.tile([128, 512], bf16, tag="wk")
                nc.sync.dma_start(wk_sb[:], wkp[:, i, j, :])
                nc.tensor.matmul(
                    cps[:],
                    lm_sb[:, i, :],
                    wk_sb[:],
                    start=(i == 0),
                    stop=False,
                )
            nc.tensor.matmul(
                cps[:], one_b[:], bk_sb[:, ts(j, 512)], start=False, stop=True
            )
            nc.scalar.copy(kernT_sb[:, ts(j, 512)], cps[:])

        # bounce through DRAM to re-layout coeffs into (k, h) conv weights
        scr = dram_pool.tile([BC, K, HCHUNKS, 128], fp32)
        nc.sync.dma_start(
            scr[:], kernT_sb[:].rearrange("p (k c h) -> p k c h", k=K, c=HCHUNKS)
        )
        coef_sb = coeff_pool.tile([K, BC * HCHUNKS, 128], fp32)
        for b in range(BC):
            for hc in range(HCHUNKS):
                nc.sync.dma_start(
                    coef_sb[:, b * HCHUNKS + hc, :], scr[b, :, hc, :]
                )
        if debug_dumps:
            nc.sync.dma_start(d_kernT, kernT_sb[:])
            nc.sync.dma_start(d_coef, coef_sb[:])

        # ---------- global phase: glob[h, b] = lm @ Wg.T + bg ----------
        glob_sb = coeff_pool.tile([128, HCHUNKS, BC], fp32)
        for hc in range(HCHUNKS):
            gps = psum_misc.tile([128, BC], fp32, tag="misc", name="gps")
            for i in range(4):
                nc.tensor.matmul(
                    gps[:],
                    wgt_sb[:, i, ts(hc, 128)],
                    lmf_sb[:, i, :],
                    start=(i == 0),
                    stop=False,
                )
            nc.tensor.matmul(
                gps[:], bg_sb[:, ts(hc, 128)], one_f[:], start=False, stop=True
            )
            nc.scalar.copy(glob_sb[:, hc, :], gps[:])
        if debug_dumps:
            nc.sync.dma_start(d_glob, glob_sb[:])

        # ---------- main stream ----------
        spsum = spsum_pool.tile([128, 128], fp32)
        for b in range(BC):
            win_sb = win_pool.tile([K, T], fp32, tag="win")
            nc.sync.dma_start(win_sb[:], win[b, :, :])
            for tt in range(NTT):
                tan_tiles = []
                for hc in range(HCHUNKS):
                    enc_sb = enc_pool.tile([128, TTILE], fp32, tag="enc")
                    nc.sync.dma_start(enc_sb[:], enc[b, hc, :, ts(tt, TTILE)])
                    cpsum = psum_pool.tile([128, TTILE], fp32, tag="conv")
                    nc.tensor.matmul(
                        cpsum[:],
                        coef_sb[:, b * HCHUNKS + hc, :],
                        win_sb[:, ts(tt, TTILE)],
                        start=True,
                        stop=True,
                    )
                    hid_sb = hid_pool.tile([128, TTILE], fp32, tag="hid")
                    nc.vector.tensor_tensor(hid_sb[:], cpsum[:], enc_sb[:], Add)
                    if debug_dumps and b == 0 and tt == 0 and hc == 0:
                        nc.sync.dma_start(d_hid, hid_sb[:])
                    tan_sb = tan_pool.tile([128, TTILE], fp32, tag="tan")
                    nc.scalar.activation(
                        tan_sb[:],
                        hid_sb[:],
                        TanhF,
                        bias=glob_sb[:, hc, b : b + 1],
                        scale=1.0,
                    )
                    if debug_dumps and b == 0 and tt == 0 and hc == 0:
                        nc.sync.dma_start(d_tan, tan_sb[:])
                    tan_tiles.append(tan_sb)
                for j in range(4):
                    col = b * 32 + tt * 4 + j
                    for hc in range(HCHUNKS):
                        nc.tensor.matmul(
                            spsum[:, col : col + 1],
                            tan_tiles[hc][:, ts(j, 128)],
                            ws_sb[:, hc : hc + 1],
                            start=(hc == 0),
                            stop=(hc == HCHUNKS - 1),
                            skip_group_check=True,
                        )

        # ---------- softmax over T (per batch column group) ----------
        sc_sb = small_pool.tile([128, 128], fp32)
        nc.vector.tensor_tensor(sc_sb[:], spsum[:], msk_sb[:], Add)
        exp_sb = small_pool.tile([128, 128], fp32)
        nc.scalar.activation(exp_sb[:], sc_sb[:], ExpF, bias=0.0, scale=1.0)
        if debug_dumps:
            nc.sync.dma_start(d_sc, sc_sb[:])
            nc.sync.dma_start(d_exp, exp_sb[:])
        red_sb = small_pool.tile([128, BC], fp32)
        nc.vector.tensor_reduce(
            red_sb[:],
            exp_sb[:].rearrange("p (b t) -> p b t", b=BC),
            mybir.AxisListType.X,
            Add,
        )
        tpsum = psum_misc.tile([1, BC], fp32, tag="misc", name="tpsum")
        nc.tensor.matmul(tpsum[:], ones128[:], red_sb[:], start=True, stop=True)
        rec_sb = small_pool.tile([1, BC], fp32)
        nc.vector.reciprocal(rec_sb[:], tpsum[:])
        bpsum = psum_misc.tile([128, BC], fp32, tag="misc", name="bpsum")
        nc.tensor.matmul(bpsum[:], ones1x128[:], rec_sb[:], start=True, stop=True)
        rb_sb = small_pool.tile([128, BC], fp32)
        nc.scalar.copy(rb_sb[:], bpsum[:])
        att_sb = small_pool.tile([128, 128], fp32)
        for b in range(BC):
            nc.vector.tensor_scalar_mul(
                att_sb[:, ts(b, 32)], exp_sb[:, ts(b, 32)], rb_sb[:, b : b + 1]
            )
        nc.sync.dma_start(att, att_sb[:])

    nc.compile()
    return nc


def _get_program():
    if "nc" not in _CACHE:
        _CACHE["nc"] = _build_program()
    return _CACHE["nc"]


def _prep_inputs(encoded_contribution, mask, lm_state, prev_att_weights,
                 Wk, bk, Wg, bg, Ws, bs):
    """Host-side shard + layout prep. Returns list of per-core input dicts."""
    f32 = np.float32
    bf16 = ml_dtypes.bfloat16

    enc = np.asarray(encoded_contribution, dtype=f32)
    mask = np.asarray(mask, dtype=f32)
    lm = np.asarray(lm_state, dtype=f32)
    prev = np.asarray(prev_att_weights, dtype=f32)
    Wk = np.asarray(Wk, dtype=f32)
    bk = np.asarray(bk, dtype=f32)
    Wg = np.asarray(Wg, dtype=f32)
    bg = np.asarray(bg, dtype=f32)
    Ws = np.asarray(Ws, dtype=f32)
    bs = np.asarray(bs, dtype=f32)

    # enc: (T, B, H) -> (B, H, T) -> (NCORES, BC, HCHUNKS, 128, T)
    enc_t = np.ascontiguousarray(enc.transpose(1, 2, 0)).reshape(
        NCORES, BC, HCHUNKS, 128, T
    )

    # toeplitz windows: win[b, k, t] = prev_pad[b, k + t]
    prev_pad = np.zeros((B, T + K - 1), dtype=f32)
    prev_pad[:, K - 1 :] = prev.T
    win_full = np.lib.stride_tricks.sliding_window_view(prev_pad, T, axis=1)
    # (B, K, T)
    win_full = win_full.reshape(NCORES, BC, K, T)

    # WkP[l, k*256+h] = Wk[h*32+k, l]; dram layout (128 p, 4 i, 16 j, 512 nn)
    wkp = (
        Wk.reshape(H, K, L)
        .transpose(2, 1, 0)          # (L, K, H)
        .reshape(L, K * H)
        .astype(bf16)
        .reshape(4, 128, 16, 512)
        .transpose(1, 0, 2, 3)
    )
    wkp = np.ascontiguousarray(wkp)

    # bk permuted to [k*256+h]
    bkp = np.ascontiguousarray(bk.reshape(H, K).T.reshape(1, K * H)).astype(bf16)

    # lmT chunks: (128, 4, B) sliced per core
    lmT = np.ascontiguousarray(lm.T.reshape(4, 128, B).transpose(1, 0, 2))
    # (128, 4, B)

    # WgT chunks: (128, 4, H)
    wgt = np.ascontiguousarray(Wg.T.reshape(4, 128, H).transpose(1, 0, 2))

    bgp = np.ascontiguousarray(bg.reshape(1, H))
    wsv = np.ascontiguousarray(Ws[0].reshape(HCHUNKS, 128).T)

    in_maps = []
    for c in range(NCORES):
        m = mask[:, c * BC : (c + 1) * BC] + bs[0]
        # msk[p, b*32 + tt*4 + j] = m[tt*512 + j*128 + p, b]
        mskc = np.ascontiguousarray(
            m.reshape(NTT, 4, 128, BC).transpose(2, 3, 0, 1).reshape(128, 128)
        )
        lmc = np.ascontiguousarray(lmT[:, :, c * BC : (c + 1) * BC])
        in_maps.append(
            {
                "enc": np.ascontiguousarray(enc_t[c]),
                "win": np.ascontiguousarray(win_full[c]),
                "wkp": wkp,
                "bkp": bkp,
                "lmb": lmc.astype(bf16),
                "lmf": lmc,
                "wgt": wgt,
                "bgp": bgp,
                "wsv": wsv,
                "msk": mskc,
            }
        )
    return in_maps


def _assemble_output(per_core):
    out = np.empty((T, B), dtype=np.float32)
    for c in range(NCORES):
        A = np.asarray(per_core[c], dtype=np.float32)
        # A[p, b*32 + tt*4 + j] = att[tt*512 + j*128 + p, c*BC + b]
        blk = A.reshape(128, BC, NTT, 4).transpose(2, 3, 0, 1).reshape(T, BC)
        out[:, c * BC : (c + 1) * BC] = blk
    return out


def kernel(**inputs):
    from concourse.bass_utils import run_bass_kernel_spmd

    in_maps = _prep_inputs(**inputs)
    nc = _get_program()
    trace = bool(os.environ.get("BASS_TRACE"))
    res = run_bass_kernel_spmd(nc, in_maps, list(range(NCORES)), trace=trace)
    _CACHE["last_results"] = res
    return _assemble_output([r["att"] for r in res.results])


# revision 12
# speedup vs baseline: 2.4201x; 2.4201x over previous
"""Trainium2 Bass kernel for nn_LocalAttention (T=4096, B=32, H=256, L=512, K=32).

Sharding: data-parallel over batch B across 8 cores (4 batch elements per core).

Per-core dataflow (h-on-partitions layout):
  1. coeff phase: kernel coeffs = lm @ Wk.T + bk computed on PE from a
     host-permuted Wk (bf16), landing as (4 batches, 8192) in PSUM chunks,
     copied to SBUF, bounced through DRAM to re-layout into (32 k, 128 h)
     conv weight tiles.
  2. global phase: global = lm @ Wg.T + bg on PE -> (128 h, 4 b) SBUF, used as
     the per-partition bias of the tanh activation.
  3. main stream per (b, hc, tt): conv = coeff.T @ toeplitz(prev) on PE into
     PSUM (128 h, 512 t); DVE adds encoded_contribution; ACT computes
     tanh(. + global_bias); PE reduces over h against Ws -> scores in
     (128 t-part, col) layout; softmax over T on-chip.
"""

import os
import sys

import numpy as np

if "/opt/trn_rl_repo" not in sys.path:
    sys.path.insert(0, "/opt/trn_rl_repo")

import ml_dtypes

T, B, H, L, K = 4096, 32, 256, 512, 32
NCORES = 8
BC = B // NCORES          # 4 batches per core
HCHUNKS = H // 128        # 2
TTILE = 512
NTT = T // TTILE          # 8 t-tiles per (b, hc)

_CACHE = {}


def _build_program(debug_dumps=False):
    import concourse.bacc as bacc
    import concourse.bass as bass
    import concourse.mybir as mybir
    import concourse.tile as tile
    from contextlib import ExitStack

    dt = mybir.dt
    fp32 = dt.float32
    bf16 = dt.bfloat16
    fp16 = dt.float16
    ts = bass.ts

    nc = bacc.Bacc(
        "TRN2",
        target_bir_lowering=False,
        debug=False,
        enable_asserts=False,
        num_devices=NCORES,
    )

    enc = nc.dram_tensor("enc", (BC, HCHUNKS, 128, T), fp16, kind="ExternalInput").ap()
    win = nc.dram_tensor("win", (BC, K, T), fp16, kind="ExternalInput").ap()
    wkp = nc.dram_tensor("wkp", (128, 4, 16, 512), bf16, kind="ExternalInput").ap()
    bkp = nc.dram_tensor("bkp", (1, K * H), bf16, kind="ExternalInput").ap()
    lmb = nc.dram_tensor("lmb", (128, 4, BC), bf16, kind="ExternalInput").ap()
    lmf = nc.dram_tensor("lmf", (128, 4, BC), fp32, kind="ExternalInput").ap()
    wgt = nc.dram_tensor("wgt", (128, 4, H), fp32, kind="ExternalInput").ap()
    bgp = nc.dram_tensor("bgp", (1, H), fp32, kind="ExternalInput").ap()
    wsv = nc.dram_tensor("wsv", (128, HCHUNKS), fp16, kind="ExternalInput").ap()
    msk = nc.dram_tensor("msk", (128, 128), fp32, kind="ExternalInput").ap()
    att = nc.dram_tensor("att", (128, 128), fp32, kind="ExternalOutput").ap()
    if debug_dumps:
        d_kernT = nc.dram_tensor("d_kernT", (BC, K * H), fp16, kind="ExternalOutput").ap()
        d_coef = nc.dram_tensor("d_coef", (K, BC * HCHUNKS, 128), fp16, kind="ExternalOutput").ap()
        d_glob = nc.dram_tensor("d_glob", (128, HCHUNKS, BC), fp32, kind="ExternalOutput").ap()
        d_hid = nc.dram_tensor("d_hid", (128, TTILE), fp32, kind="ExternalOutput").ap()
        d_tan = nc.dram_tensor("d_tan", (128, TTILE), fp16, kind="ExternalOutput").ap()
        d_sc = nc.dram_tensor("d_sc", (128, 128), fp32, kind="ExternalOutput").ap()
        d_exp = nc.dram_tensor("d_exp", (128, 128), fp32, kind="ExternalOutput").ap()

    TanhF = mybir.ActivationFunctionType.Tanh
    ExpF = mybir.ActivationFunctionType.Exp
    Add = mybir.AluOpType.add

    with tile.TileContext(nc) as tc, ExitStack() as ctx:
        # ---------- pools ----------
        coeff_pool = ctx.enter_context(tc.tile_pool(name="coeff", bufs=1))
        wk_pool = ctx.enter_context(tc.tile_pool(name="wkpool", bufs=6))
        small_pool = ctx.enter_context(tc.tile_pool(name="small", bufs=1))
        enc_pool = ctx.enter_context(tc.tile_pool(name="encp", bufs=4))
        win_pool = ctx.enter_context(tc.tile_pool(name="winp", bufs=2))
        hid_pool = ctx.enter_context(tc.tile_pool(name="hidp", bufs=4))
        tan_pool = ctx.enter_context(tc.tile_pool(name="tanp", bufs=4))
        psum_pool = ctx.enter_context(tc.tile_pool(name="psum", bufs=4, space="PSUM"))
        psum_cps = ctx.enter_context(tc.tile_pool(name="psumc", bufs=2, space="PSUM"))
        psum_misc = ctx.enter_context(tc.tile_pool(name="psumm", bufs=1, space="PSUM"))
        spsum_pool = ctx.enter_context(tc.tile_pool(name="spsum", bufs=1, space="PSUM"))
        dram_pool = ctx.enter_context(tc.tile_pool(name="dramp", bufs=1, space="DRAM"))

        # ---------- small input loads ----------
        lm_sb = small_pool.tile([128, 4, BC], bf16)
        nc.sync.dma_start(lm_sb[:], lmb)
        lmf_sb = small_pool.tile([128, 4, BC], fp32)
        nc.sync.dma_start(lmf_sb[:], lmf)
        wgt_sb = small_pool.tile([128, 4, H], fp32)
        nc.sync.dma_start(wgt_sb[:], wgt)
        bk_sb = small_pool.tile([1, K * H], bf16)
        nc.sync.dma_start(bk_sb[:], bkp)
        bg_sb = small_pool.tile([1, H], fp32)
        nc.sync.dma_start(bg_sb[:], bgp)
        ws_sb = small_pool.tile([128, HCHUNKS], fp16)
        nc.sync.dma_start(ws_sb[:], wsv)
        msk_sb = small_pool.tile([128, 128], fp32)
        nc.sync.dma_start(msk_sb[:], msk)

        one_b = small_pool.tile([1, BC], bf16)
        nc.vector.memset(one_b[:], 1.0)
        one_f = small_pool.tile([1, BC], fp32)
        nc.vector.memset(one_f[:], 1.0)
        ones128 = small_pool.tile([128, 1], fp32)
        nc.vector.memset(ones128[:], 1.0)
        ones1x128 = small_pool.tile([1, 128], fp32)
        nc.vector.memset(ones1x128[:], 1.0)

        # ---------- coeff phase: kernT[b, k*256+h] = (lm @ WkP + bk) ----------
        kernT_sb = coeff_pool.tile([BC, K * H], fp16)
        for j in range(16):
            cps = psum_cps.tile([BC, 512], fp32, tag="cps")
            for i in range(4):
                wk_sb = wk_pool.tile([128, 512], bf16, tag="wk")
                nc.sync.dma_start(wk_sb[:], wkp[:, i, j, :])
                nc.tensor.matmul(
                    cps[:],
                    lm_sb[:, i, :],
                    wk_sb[:],
                    start=(i == 0),
                    stop=False,
                )
            nc.tensor.matmul(
                cps[:], one_b[:], bk_sb[:, ts(j, 512)], start=False, stop=True
            )
            nc.scalar.copy(kernT_sb[:, ts(j, 512)], cps[:])

        # bounce through DRAM to re-layout coeffs into (k, h) conv weights
        scr = dram_pool.tile([BC, K, HCHUNKS, 128], fp16)
        nc.sync.dma_start(
            scr[:], kernT_sb[:].rearrange("p (k c h) -> p k c h", k=K, c=HCHUNKS)
        )
        coef_sb = coeff_pool.tile([K, BC * HCHUNKS, 128], fp16)
        for b in range(BC):
            for hc in range(HCHUNKS):
                nc.sync.dma_start(
                    coef_sb[:, b * HCHUNKS + hc, :], scr[b, :, hc, :]
                )
        if debug_dumps:
            nc.sync.dma_start(d_kernT, kernT_sb[:])
            nc.sync.dma_start(d_coef, coef_sb[:])

        # ---------- global phase: glob[h, b] = lm @ Wg.T + bg ----------
        glob_sb = coeff_pool.tile([128, HCHUNKS, BC], fp32)
        for hc in range(HCHUNKS):
            gps = psum_misc.tile([128, BC], fp32, tag="misc", name="gps")
            for i in range(4):
                nc.tensor.matmul(
                    gps[:],
                    wgt_sb[:, i, ts(hc, 128)],
                    lmf_sb[:, i, :],
                    start=(i == 0),
                    stop=False,
                )
            nc.tensor.matmul(
                gps[:], bg_sb[:, ts(hc, 128)], one_f[:], start=False, stop=True
            )
            nc.scalar.copy(glob_sb[:, hc, :], gps[:])
        if debug_dumps:
            nc.sync.dma_start(d_glob, glob_sb[:])

        # ---------- main stream ----------
        spsum = spsum_pool.tile([128, 128], fp32)
        for b in range(BC):
            win_sb = win_pool.tile([K, T], fp16, tag="win")
            nc.sync.dma_start(win_sb[:], win[b, :, :])
            enc_tiles = []
            for hc in range(HCHUNKS):
                enc_sb = enc_pool.tile([128, T], fp16, tag="enc")
                nc.sync.dma_start(enc_sb[:], enc[b, hc, :, :])
                enc_tiles.append(enc_sb)
            for tt in range(NTT):
                tan_tiles = []
                for hc in range(HCHUNKS):
                    cpsum = psum_pool.tile([128, TTILE], fp32, tag="conv")
                    nc.tensor.matmul(
                        cpsum[:],
                        coef_sb[:, b * HCHUNKS + hc, :],
                        win_sb[:, ts(tt, TTILE)],
                        start=True,
                        stop=True,
                    )
                    hid_sb = hid_pool.tile([128, TTILE], fp32, tag="hid")
                    nc.vector.tensor_tensor(hid_sb[:], cpsum[:], enc_tiles[hc][:, ts(tt, TTILE)], Add)
                    if debug_dumps and b == 0 and tt == 0 and hc == 0:
                        nc.sync.dma_start(d_hid, hid_sb[:])
                    tan_sb = tan_pool.tile([128, TTILE], fp16, tag="tan")
                    nc.scalar.activation(
                        tan_sb[:],
                        hid_sb[:],
                        TanhF,
                        bias=glob_sb[:, hc, b : b + 1],
                        scale=1.0,
                    )
                    if debug_dumps and b == 0 and tt == 0 and hc == 0:
                        nc.sync.dma_start(d_tan, tan_sb[:])
                    tan_tiles.append(tan_sb)
                for j in range(4):
                    col = b * 32 + tt * 4 + j
                    for hc in range(HCHUNKS):
                        nc.tensor.matmul(
                            spsum[:, col : col + 1],
                            tan_tiles[hc][:, ts(j, 128)],
                            ws_sb[:, hc : hc + 1],
                            start=(hc == 0),
                            stop=(hc == HCHUNKS - 1),
                            skip_group_check=True,
                        )

        # ---------- softmax over T (per batch column group) ----------
        sc_sb = small_pool.tile([128, 128], fp32)
        nc.vector.tensor_tensor(sc_sb[:], spsum[:], msk_sb[:], Add)
        exp_sb = small_pool.tile([128, 128], fp32)
        nc.scalar.activation(exp_sb[:], sc_sb[:], ExpF, bias=0.0, scale=1.0)
        if debug_dumps:
            nc.sync.dma_start(d_sc, sc_sb[:])
            nc.sync.dma_start(d_exp, exp_sb[:])
        red_sb = small_pool.tile([128, BC], fp32)
        nc.vector.tensor_reduce(
            red_sb[:],
            exp_sb[:].rearrange("p (b t) -> p b t", b=BC),
            mybir.AxisListType.X,
            Add,
        )
        tpsum = psum_misc.tile([1, BC], fp32, tag="misc", name="tpsum")
        nc.tensor.matmul(tpsum[:], ones128[:], red_sb[:], start=True, stop=True)
        rec_sb = small_pool.tile([1, BC], fp32)
        nc.vector.reciprocal(rec_sb[:], tpsum[:])
        bpsum = psum_misc.tile([128, BC], fp32, tag="misc", name="bpsum")
        nc.tensor.matmul(bpsum[:], ones1x128[:], rec_sb[:], start=True, stop=True)
        rb_sb = small_pool.tile([128, BC], fp32)
        nc.scalar.copy(rb_sb[:], bpsum[:])
        att_sb = small_pool.tile([128, 128], fp32)
        for b in range(BC):
            nc.vector.tensor_scalar_mul(
                att_sb[:, ts(b, 32)], exp_sb[:, ts(b, 32)], rb_sb[:, b : b + 1]
            )
        nc.sync.dma_start(att, att_sb[:])

    nc.compile()
    return nc


def _get_program():
    if "nc" not in _CACHE:
        _CACHE["nc"] = _build_program()
    return _CACHE["nc"]


def _prep_inputs(encoded_contribution, mask, lm_state, prev_att_weights,
                 Wk, bk, Wg, bg, Ws, bs):
    """Host-side shard + layout prep. Returns list of per-core input dicts."""
    f32 = np.float32
    bf16 = ml_dtypes.bfloat16

    enc = np.asarray(encoded_contribution, dtype=f32)
    mask = np.asarray(mask, dtype=f32)
    lm = np.asarray(lm_state, dtype=f32)
    prev = np.asarray(prev_att_weights, dtype=f32)
    Wk = np.asarray(Wk, dtype=f32)
    bk = np.asarray(bk, dtype=f32)
    Wg = np.asarray(Wg, dtype=f32)
    bg = np.asarray(bg, dtype=f32)
    Ws = np.asarray(Ws, dtype=f32)
    bs = np.asarray(bs, dtype=f32)

    # enc: (T, B, H) -> (B, H, T) -> (NCORES, BC, HCHUNKS, 128, T)
    enc_t = np.ascontiguousarray(enc.transpose(1, 2, 0).astype(np.float16)).reshape(
        NCORES, BC, HCHUNKS, 128, T
    )

    # toeplitz windows: win[b, k, t] = prev_pad[b, k + t]
    prev_pad = np.zeros((B, T + K - 1), dtype=f32)
    prev_pad[:, K - 1 :] = prev.T
    win_full = np.lib.stride_tricks.sliding_window_view(prev_pad, T, axis=1)
    # (B, K, T)
    win_full = win_full.astype(np.float16).reshape(NCORES, BC, K, T)

    # WkP[l, k*256+h] = Wk[h*32+k, l]; dram layout (128 p, 4 i, 16 j, 512 nn)
    wkp = (
        Wk.reshape(H, K, L)
        .transpose(2, 1, 0)          # (L, K, H)
        .reshape(L, K * H)
        .astype(bf16)
        .reshape(4, 128, 16, 512)
        .transpose(1, 0, 2, 3)
    )
    wkp = np.ascontiguousarray(wkp)

    # bk permuted to [k*256+h]
    bkp = np.ascontiguousarray(bk.reshape(H, K).T.reshape(1, K * H)).astype(bf16)

    # lmT chunks: (128, 4, B) sliced per core
    lmT = np.ascontiguousarray(lm.T.reshape(4, 128, B).transpose(1, 0, 2))
    # (128, 4, B)

    # WgT chunks: (128, 4, H)
    wgt = np.ascontiguousarray(Wg.T.reshape(4, 128, H).transpose(1, 0, 2))

    bgp = np.ascontiguousarray(bg.reshape(1, H))
    wsv = np.ascontiguousarray(Ws[0].reshape(HCHUNKS, 128).T).astype(np.float16)

    in_maps = []
    for c in range(NCORES):
        m = mask[:, c * BC : (c + 1) * BC] + bs[0]
        # msk[p, b*32 + tt*4 + j] = m[tt*512 + j*128 + p, b]
        mskc = np.ascontiguousarray(
            m.reshape(NTT, 4, 128, BC).transpose(2, 3, 0, 1).reshape(128, 128)
        )
        lmc = np.ascontiguousarray(lmT[:, :, c * BC : (c + 1) * BC])
        in_maps.append(
            {
                "enc": np.ascontiguousarray(enc_t[c]),
                "win": np.ascontiguousarray(win_full[c]),
                "wkp": wkp,
                "bkp": bkp,
                "lmb": lmc.astype(bf16),
                "lmf": lmc,
                "wgt": wgt,
                "bgp": bgp,
                "wsv": wsv,
                "msk": mskc,
            }
        )
    return in_maps


def _assemble_output(per_core):
    out = np.empty((T, B), dtype=np.float32)
    for c in range(NCORES):
        A = np.asarray(per_core[c], dtype=np.float32)
        # A[p, b*32 + tt*4 + j] = att[tt*512 + j*128 + p, c*BC + b]
        blk = A.reshape(128, BC, NTT, 4).transpose(2, 3, 0, 1).reshape(T, BC)
        out[:, c * BC : (c + 1) * BC] = blk
    return out


def kernel(**inputs):
    from concourse.bass_utils import run_bass_kernel_spmd

    in_maps = _prep_inputs(**inputs)
    nc = _get_program()
    trace = bool(os.environ.get("BASS_TRACE"))
    res = run_bass_kernel_spmd(nc, in_maps, list(range(NCORES)), trace=trace)
    _CACHE["last_results"] = res
    return _assemble_output([r["att"] for r in res.results])
